# revision 13
# baseline (speedup 1.0000x reference)
"""Trilinear scatter-add (splat) + Huber loss kernel for Trainium2, 8 NeuronCores.

Strategy
--------
reference computes:  huber_sum(splat(coords+pred) - splat(coords+gt)) over a
128^3 grid with trilinear weights and vals=1.

Key identity: the trilinear corner weights of a point with pixel coordinate p
along one axis are exactly  hat_j(p) = relu(1 - |p - j|)  for bin j in [0,128):
two adjacent nonzeros (1-frac, frac), and out-of-range corners drop out
automatically (matching grid_sample's zeros padding).

So for a batch of K points, the (y,x)-plane contribution at a fixed z-plane is
a dense matmul:   plane[y,x] += sum_k  wz_k * hat(y_k - y) * hat(x_k - x)
                             = (Wz.hatY)^T @ hatX

Sharding: the host (inside kernel(), as the sharding step) bins points by
z0 = floor(z_pix) into 129 groups (-1..127) and assigns core c the z-planes
[16c, 16c+15].  Core c processes groups 16c-1 .. 16c+15: group g contributes
to plane g (weight 1-fz) and plane g+1 (weight fz).  Each group is padded to a
fixed size so the single SPMD program works for every core; padded records use
far-away coordinates so every hat weight is exactly 0.

On device, per tile of 128 points (points live in partitions):
  DVE:  absd_y = |iota - y|;  hm_y = min(absd_y-1, 0) = -hat_y
        absd_x = |iota - x|;  X = min(absd_x-1, 0)    = -hat_x      (bf16)
  ACT:  A = hm_y * (fz-1)  = hat_y*(1-fz)   (bf16)
        B = hm_y * (-fz)   = hat_y*fz       (bf16)
  PE :  pair[x, 0:128 | 128:256] += X^T @ [A | B]   (PSUM f32 accumulate)
(The produced planes are globally negated; Huber is symmetric so it cancels.)

Plane p is then  pairs[p][:, :128] + pairs[p-1][:, 128:];  Huber uses the
branch-free identity  huber(d) = m*(|d| - m/2),  m = min(|d|, 1).
Per-core output is a [128] vector of partial sums; host adds them up.
"""

import os
import sys
import numpy as np

sys.path.insert(0, "/opt/trn_rl_repo")

from contextlib import ExitStack

import concourse.bass as bass
import concourse.tile as tile
from concourse import bacc, mybir
from concourse.bass_utils import run_bass_kernel_spmd

F32 = mybir.dt.float32
BF16 = mybir.dt.bfloat16

D = H = W = 128
N_CORES = 8
NG = 17            # groups per core: z0 in [16c-1, 16c+15]
NT_MIN = 8         # tiles of 128 points per group; actual NT sized from data

NREC = 8
_PAD_REC = np.array([40959.0, -40961.0, 40959.0, -40961.0, 0.0, 0.0,
                     40960.0, 40960.0], dtype=np.float32)


def _pix_groups(pts: np.ndarray):
    x = ((pts[:, 0] + 1.0) * np.float32(W) - 1.0) * np.float32(0.5)
    y = ((pts[:, 1] + 1.0) * np.float32(H) - 1.0) * np.float32(0.5)
    z = ((pts[:, 2] + 1.0) * np.float32(D) - 1.0) * np.float32(0.5)
    z0 = np.floor(z).astype(np.int32)
    keep = (z0 >= -1) & (z0 <= 127)
    return x[keep], y[keep], z[keep], z0[keep]


def _route_points(pts: np.ndarray, nt: int):
    """pts [N,3] float32 -> per-core [NG, 128, nt*NREC] float32 record arrays."""
    g_pad = nt * 128
    x, y, z, z0 = _pix_groups(pts)
    fz = z - z0.astype(np.float32)
    # record = [ym1, nym1, xm1, nxm1, fz-1, -fz, y, x]
    #   e = (iota * -1) + ym1 = y-1-iota ;  f = iota + nym1 = iota-y-1
    #   g = max(e, f) = |iota-y| - 1 ;  A = min(g,0)*(fz-1) = hat_y*(1-fz)
    recs = np.stack([y - 1.0, -y - 1.0, x - 1.0, -x - 1.0,
                     fz - 1.0, -fz, y, x], axis=1).astype(np.float32)

    order = np.argsort(z0, kind="stable")
    z0s = z0[order]
    recs_s = recs[order]
    counts = np.bincount(z0s + 1, minlength=129)
    if counts.max() > g_pad:
        raise RuntimeError(f"group overflow: {counts.max()} > {g_pad}")
    starts = np.concatenate([[0], np.cumsum(counts)])

    glob = np.empty((129, g_pad, NREC), dtype=np.float32)
    glob[:] = _PAD_REC
    for g in range(129):
        n = counts[g]
        if n:
            glob[g, :n] = recs_s[starts[g]:starts[g] + n]

    per_core = []
    for c in range(N_CORES):
        arr = glob[16 * c: 16 * c + NG]                       # [NG, g_pad, NREC]
        arr = arr.reshape(NG, nt, 128, NREC).transpose(0, 2, 1, 3)
        per_core.append(np.ascontiguousarray(arr.reshape(NG, 128, nt * NREC)))
    return per_core


def build_bass(ng, nt):
    nc = bacc.Bacc(
        "TRN2", target_bir_lowering=False, debug=False, num_devices=N_CORES)
    recs_p = nc.declare_dram_parameter("recs_pred", [ng, 128, nt * NREC], F32, isOutput=False)
    recs_g = nc.declare_dram_parameter("recs_gt", [ng, 128, nt * NREC], F32, isOutput=False)
    out_part = nc.declare_dram_parameter("partials", [128, 1], F32, isOutput=True)

    iota_np = np.tile(np.arange(128, dtype=np.float32), (128, 1))
    iota_dram = nc.inline_tensor(iota_np, "iota_const")

    recs_in = {0: recs_p, 1: recs_g}

    with tile.TileContext(nc) as tc, ExitStack() as ctx:
        const_pool = ctx.enter_context(tc.tile_pool(name="const", bufs=1))
        rec_pool = ctx.enter_context(tc.tile_pool(name="recs", bufs=4))
        work_pool = ctx.enter_context(tc.tile_pool(name="work", bufs=4))
        ab_pool = ctx.enter_context(tc.tile_pool(name="ab", bufs=4))
        x_pool = ctx.enter_context(tc.tile_pool(name="xt", bufs=4))
        flush_pool = ctx.enter_context(tc.tile_pool(name="flush", bufs=2))
        acc_pool = ctx.enter_context(tc.tile_pool(name="acc", bufs=1))
        psum_pools = {
            0: ctx.enter_context(tc.tile_pool(name="psum_p", bufs=3, space="PSUM")),
            1: ctx.enter_context(tc.tile_pool(name="psum_g", bufs=3, space="PSUM")),
        }

        iota_sb = const_pool.tile([128, 128], F32)
        nc.sync.dma_start(iota_sb[:], iota_dram[:])

        acc = acc_pool.tile([128, 128], F32)
        nc.vector.memset(acc[:], 0.0)

        pairs = {0: {}, 1: {}}  # grid -> local group idx -> psum pair tile

        for gi in range(ng):
            for grid in (0, 1):
                rec = rec_pool.tile([128, nt * NREC], F32, tag="rec")
                nc.sync.dma_start(rec[:], recs_in[grid][gi])

                pair = psum_pools[grid].tile([128, 256], F32, tag="pair")
                pairs[grid][gi] = pair

                for t in range(nt):
                    ym1 = rec[:, NREC * t + 0: NREC * t + 1]
                    nym1 = rec[:, NREC * t + 1: NREC * t + 2]
                    xm1 = rec[:, NREC * t + 2: NREC * t + 3]
                    nxm1 = rec[:, NREC * t + 3: NREC * t + 4]
                    fa_col = rec[:, NREC * t + 4: NREC * t + 5]
                    fb_col = rec[:, NREC * t + 5: NREC * t + 6]

                    e_y = work_pool.tile([128, 128], F32, tag="e_y")
                    nc.vector.tensor_scalar(
                        e_y[:], iota_sb[:], -1.0, ym1,
                        mybir.AluOpType.mult, mybir.AluOpType.add)
                    f_y = work_pool.tile([128, 128], F32, tag="f_y")
                    nc.vector.tensor_scalar(
                        f_y[:], iota_sb[:], nym1, None, mybir.AluOpType.add)
                    g_y = work_pool.tile([128, 128], F32, tag="g_y")
                    nc.vector.tensor_tensor(g_y[:], e_y[:], f_y[:], mybir.AluOpType.max)

                    ab = ab_pool.tile([128, 256], BF16, tag="ab")
                    nc.vector.tensor_scalar(
                        ab[:, 0:128], g_y[:], 0.0, fa_col,
                        mybir.AluOpType.min, mybir.AluOpType.mult)
                    nc.vector.tensor_scalar(
                        ab[:, 128:256], g_y[:], 0.0, fb_col,
                        mybir.AluOpType.min, mybir.AluOpType.mult)

                    e_x = work_pool.tile([128, 128], F32, tag="e_x")
                    nc.vector.tensor_scalar(
                        e_x[:], iota_sb[:], -1.0, xm1,
                        mybir.AluOpType.mult, mybir.AluOpType.add)
                    f_x = work_pool.tile([128, 128], F32, tag="f_x")
                    nc.vector.tensor_scalar(
                        f_x[:], iota_sb[:], nxm1, None, mybir.AluOpType.add)
                    g_x = work_pool.tile([128, 128], F32, tag="g_x")
                    nc.vector.tensor_tensor(g_x[:], e_x[:], f_x[:], mybir.AluOpType.max)
                    xt = x_pool.tile([128, 128], BF16, tag="xt")
                    nc.vector.tensor_scalar(
                        xt[:], g_x[:], 0.0, -1.0,
                        mybir.AluOpType.min, mybir.AluOpType.mult)

                    nc.tensor.matmul(
                        pair[:], xt[:], ab[:],
                        start=(t == 0), stop=(t == nt - 1))

            # flush local plane gi (valid for gi >= 1)
            if gi >= 1:
                pP1, pP0 = pairs[0][gi], pairs[0][gi - 1]
                pG1, pG0 = pairs[1][gi], pairs[1][gi - 1]
                c1 = flush_pool.tile([128, 128], F32, tag="c1")
                nc.scalar.copy(c1[:], pP1[:, 0:128])
                t1 = flush_pool.tile([128, 128], F32, tag="t1")
                nc.vector.tensor_tensor(t1[:], c1[:], pP0[:, 128:256], mybir.AluOpType.add)
                c2 = flush_pool.tile([128, 128], F32, tag="c2")
                nc.scalar.copy(c2[:], pG1[:, 0:128])
                t2 = flush_pool.tile([128, 128], F32, tag="t2")
                nc.vector.tensor_tensor(t2[:], c2[:], pG0[:, 128:256], mybir.AluOpType.add)
                d = flush_pool.tile([128, 128], F32, tag="d")
                nc.vector.tensor_tensor(d[:], t1[:], t2[:], mybir.AluOpType.subtract)
                nd = flush_pool.tile([128, 128], F32, tag="nd")
                nc.vector.tensor_scalar(
                    nd[:], d[:], -1.0, None, mybir.AluOpType.mult)
                a = flush_pool.tile([128, 128], F32, tag="a")
                nc.vector.tensor_tensor(a[:], d[:], nd[:], mybir.AluOpType.max)
                m = flush_pool.tile([128, 128], F32, tag="m")
                nc.vector.tensor_scalar(
                    m[:], a[:], 1.0, None, mybir.AluOpType.min)
                mh = flush_pool.tile([128, 128], F32, tag="mh")
                nc.vector.tensor_scalar(
                    mh[:], m[:], 0.5, None, mybir.AluOpType.mult)
                s = flush_pool.tile([128, 128], F32, tag="s")
                nc.vector.tensor_tensor(s[:], a[:], mh[:], mybir.AluOpType.subtract)
                h = flush_pool.tile([128, 128], F32, tag="h")
                nc.vector.tensor_tensor(h[:], m[:], s[:], mybir.AluOpType.mult)
                nc.vector.tensor_tensor(acc[:], acc[:], h[:], mybir.AluOpType.add)

        red = acc_pool.tile([128, 1], F32)
        nc.vector.tensor_reduce(red[:], acc[:], mybir.AxisListType.X, mybir.AluOpType.add)
        nc.sync.dma_start(out_part[:], red[:])

    nc.compile()
    return nc


_NC_CACHE = {}


def kernel(registration_pred, registration_gt, coords):
    coords = np.asarray(coords, dtype=np.float32)
    registration_pred = np.asarray(registration_pred, dtype=np.float32)
    registration_gt = np.asarray(registration_gt, dtype=np.float32)

    pred_pts = (coords + registration_pred).reshape(-1, 3).astype(np.float32)
    gt_pts = (coords + registration_gt).reshape(-1, 3).astype(np.float32)

    nt = NT_MIN
    for pts in (pred_pts, gt_pts):
        z0 = _pix_groups(pts)[3]
        nt = max(nt, int(-(-np.bincount(z0 + 1, minlength=129).max() // 128)))

    pred_cores = _route_points(pred_pts, nt)
    gt_cores = _route_points(gt_pts, nt)

    key = (NG, nt)
    if key not in _NC_CACHE:
        _NC_CACHE[key] = build_bass(NG, nt)
    nc = _NC_CACHE[key]

    in_maps = [
        {"recs_pred": pred_cores[c], "recs_gt": gt_cores[c]}
        for c in range(N_CORES)
    ]
    res = run_bass_kernel_spmd(nc, in_maps, list(range(N_CORES)))
    total = np.float64(0.0)
    for c in range(N_CORES):
        total += np.asarray(res.results[c]["partials"], dtype=np.float64).sum()
    return np.float32(total)


if __name__ == "__main__":
    rng = np.random.default_rng(0)
    n = 5000
    coords = rng.uniform(-0.95, 0.95, (1, n, 3)).astype(np.float32)
    rp = (0.02 * rng.standard_normal((1, n, 3))).astype(np.float32)
    rg = (0.02 * rng.standard_normal((1, n, 3))).astype(np.float32)
    print(kernel(rp, rg, coords))


# revision 14
# speedup vs baseline: 2.1851x; 2.1851x over previous
"""Trilinear scatter-add (splat) + Huber loss kernel for Trainium2, 8 NeuronCores.

Strategy
--------
reference computes:  huber_sum(splat(coords+pred) - splat(coords+gt)) over a
128^3 grid with trilinear weights and vals=1.

Key identity: the trilinear corner weights of a point with pixel coordinate p
along one axis are exactly  hat_j(p) = relu(1 - |p - j|)  for bin j in [0,128):
two adjacent nonzeros (1-frac, frac), and out-of-range corners drop out
automatically (matching grid_sample's zeros padding).

So for a batch of K points, the (y,x)-plane contribution at a fixed z-plane is
a dense matmul:   plane[y,x] += sum_k  wz_k * hat(y_k - y) * hat(x_k - x)
                             = (Wz.hatY)^T @ hatX

Sharding: the host (inside kernel(), as the sharding step) bins points by
z0 = floor(z_pix) into 129 groups (-1..127) and assigns core c the z-planes
[16c, 16c+15].  Core c processes groups 16c-1 .. 16c+15: group g contributes
to plane g (weight 1-fz) and plane g+1 (weight fz).  Each group is padded to a
fixed size so the single SPMD program works for every core; padded records use
far-away coordinates so every hat weight is exactly 0.

On device, per tile of 128 points (points live in partitions):
  DVE:  absd_y = |iota - y|;  hm_y = min(absd_y-1, 0) = -hat_y
        absd_x = |iota - x|;  X = min(absd_x-1, 0)    = -hat_x      (bf16)
  ACT:  A = hm_y * (fz-1)  = hat_y*(1-fz)   (bf16)
        B = hm_y * (-fz)   = hat_y*fz       (bf16)
  PE :  pair[x, 0:128 | 128:256] += X^T @ [A | B]   (PSUM f32 accumulate)
(The produced planes are globally negated; Huber is symmetric so it cancels.)

Plane p is then  pairs[p][:, :128] + pairs[p-1][:, 128:];  Huber uses the
branch-free identity  huber(d) = m*(|d| - m/2),  m = min(|d|, 1).
Per-core output is a [128] vector of partial sums; host adds them up.
"""

import os
import sys
import numpy as np

sys.path.insert(0, "/opt/trn_rl_repo")

from contextlib import ExitStack

import concourse.bass as bass
import concourse.tile as tile
from concourse import bacc, mybir
from concourse.bass_utils import run_bass_kernel_spmd

F32 = mybir.dt.float32
BF16 = mybir.dt.bfloat16

D = H = W = 128
N_CORES = 8
NG = 17            # groups per core: z0 in [16c-1, 16c+15]
NT_MIN = 8         # tiles of 128 points per group; actual NT sized from data

NREC = 4
_PAD_REC = np.array([-40960.0, -40960.0, 0.0, 0.0], dtype=np.float32)


def _pix_groups(pts: np.ndarray):
    x = ((pts[:, 0] + 1.0) * np.float32(W) - 1.0) * np.float32(0.5)
    y = ((pts[:, 1] + 1.0) * np.float32(H) - 1.0) * np.float32(0.5)
    z = ((pts[:, 2] + 1.0) * np.float32(D) - 1.0) * np.float32(0.5)
    z0 = np.floor(z).astype(np.int32)
    keep = (z0 >= -1) & (z0 <= 127)
    return x[keep], y[keep], z[keep], z0[keep]


def _route_points(pts: np.ndarray, nt: int):
    """pts [N,3] float32 -> per-core [NG, 128, nt*NREC] float32 record arrays."""
    g_pad = nt * 128
    x, y, z, z0 = _pix_groups(pts)
    fz = z - z0.astype(np.float32)
    # record = [-y, -x, 1-fz, fz]
    #   ACT Abs:  absd = |iota + (-y)| ;  DVE: hm = min(absd-1, 0) = -hat
    #   A = hm_y*(1-fz), B = hm_y*fz, X = hm_x  ->  A*X = hat_y*hat_x*(1-fz)
    recs = np.stack([-y, -x, 1.0 - fz, fz], axis=1).astype(np.float32)

    order = np.argsort(z0, kind="stable")
    z0s = z0[order]
    recs_s = recs[order]
    counts = np.bincount(z0s + 1, minlength=129)
    if counts.max() > g_pad:
        raise RuntimeError(f"group overflow: {counts.max()} > {g_pad}")
    starts = np.concatenate([[0], np.cumsum(counts)])

    glob = np.empty((129, g_pad, NREC), dtype=np.float32)
    glob[:] = _PAD_REC
    for g in range(129):
        n = counts[g]
        if n:
            glob[g, :n] = recs_s[starts[g]:starts[g] + n]

    per_core = []
    for c in range(N_CORES):
        arr = glob[16 * c: 16 * c + NG]                       # [NG, g_pad, NREC]
        arr = arr.reshape(NG, nt, 128, NREC).transpose(0, 2, 1, 3)
        per_core.append(np.ascontiguousarray(arr.reshape(NG, 128, nt * NREC)))
    return per_core


def build_bass(ng, nt):
    nc = bacc.Bacc(
        "TRN2", target_bir_lowering=False, debug=False, num_devices=N_CORES)
    recs_p = nc.declare_dram_parameter("recs_pred", [ng, 128, nt * NREC], F32, isOutput=False)
    recs_g = nc.declare_dram_parameter("recs_gt", [ng, 128, nt * NREC], F32, isOutput=False)
    out_part = nc.declare_dram_parameter("partials", [128, 1], F32, isOutput=True)

    iota_np = np.tile(np.arange(128, dtype=np.float32), (128, 1))
    iota_dram = nc.inline_tensor(iota_np, "iota_const")

    recs_in = {0: recs_p, 1: recs_g}

    with tile.TileContext(nc) as tc, ExitStack() as ctx:
        const_pool = ctx.enter_context(tc.tile_pool(name="const", bufs=1))
        rec_pool = ctx.enter_context(tc.tile_pool(name="recs", bufs=4))
        work_pool = ctx.enter_context(tc.tile_pool(name="work", bufs=4))
        ab_pool = ctx.enter_context(tc.tile_pool(name="ab", bufs=4))
        x_pool = ctx.enter_context(tc.tile_pool(name="xt", bufs=4))
        flush_pool = ctx.enter_context(tc.tile_pool(name="flush", bufs=2))
        acc_pool = ctx.enter_context(tc.tile_pool(name="acc", bufs=1))
        psum_pools = {
            0: ctx.enter_context(tc.tile_pool(name="psum_p", bufs=3, space="PSUM")),
            1: ctx.enter_context(tc.tile_pool(name="psum_g", bufs=3, space="PSUM")),
        }

        iota_sb = const_pool.tile([128, 128], F32)
        nc.sync.dma_start(iota_sb[:], iota_dram[:])

        acc = acc_pool.tile([128, 128], F32)
        nc.vector.memset(acc[:], 0.0)

        pairs = {0: {}, 1: {}}  # grid -> local group idx -> psum pair tile

        for gi in range(ng):
            for grid in (0, 1):
                rec = rec_pool.tile([128, nt * NREC], F32, tag="rec")
                nc.sync.dma_start(rec[:], recs_in[grid][gi])

                pair = psum_pools[grid].tile([128, 256], F32, tag="pair")
                pairs[grid][gi] = pair

                for t in range(nt):
                    ny_col = rec[:, NREC * t + 0: NREC * t + 1]
                    nx_col = rec[:, NREC * t + 1: NREC * t + 2]
                    fa_col = rec[:, NREC * t + 2: NREC * t + 3]
                    fb_col = rec[:, NREC * t + 3: NREC * t + 4]

                    absd_y = work_pool.tile([128, 128], F32, tag="absd_y")
                    nc.scalar.activation(
                        absd_y[:], iota_sb[:],
                        mybir.ActivationFunctionType.Abs, bias=ny_col)
                    hm_y = work_pool.tile([128, 128], F32, tag="hm_y")
                    nc.vector.tensor_scalar(
                        hm_y[:], absd_y[:], 1.0, 0.0,
                        mybir.AluOpType.subtract, mybir.AluOpType.min)

                    ab = ab_pool.tile([128, 256], BF16, tag="ab")
                    nc.vector.tensor_scalar(
                        ab[:, 0:128], hm_y[:], fa_col, None, mybir.AluOpType.mult)
                    nc.vector.tensor_scalar(
                        ab[:, 128:256], hm_y[:], fb_col, None, mybir.AluOpType.mult)

                    absd_x = work_pool.tile([128, 128], F32, tag="absd_x")
                    nc.scalar.activation(
                        absd_x[:], iota_sb[:],
                        mybir.ActivationFunctionType.Abs, bias=nx_col)
                    xt = x_pool.tile([128, 128], BF16, tag="xt")
                    nc.vector.tensor_scalar(
                        xt[:], absd_x[:], 1.0, 0.0,
                        mybir.AluOpType.subtract, mybir.AluOpType.min)

                    nc.tensor.matmul(
                        pair[:], xt[:], ab[:],
                        start=(t == 0), stop=(t == nt - 1))

            # flush local plane gi (valid for gi >= 1)
            if gi >= 1:
                pP1, pP0 = pairs[0][gi], pairs[0][gi - 1]
                pG1, pG0 = pairs[1][gi], pairs[1][gi - 1]
                c1 = flush_pool.tile([128, 128], F32, tag="c1")
                nc.scalar.copy(c1[:], pP1[:, 0:128])
                t1 = flush_pool.tile([128, 128], F32, tag="t1")
                nc.vector.tensor_tensor(t1[:], c1[:], pP0[:, 128:256], mybir.AluOpType.add)
                c2 = flush_pool.tile([128, 128], F32, tag="c2")
                nc.scalar.copy(c2[:], pG1[:, 0:128])
                t2 = flush_pool.tile([128, 128], F32, tag="t2")
                nc.vector.tensor_tensor(t2[:], c2[:], pG0[:, 128:256], mybir.AluOpType.add)
                d = flush_pool.tile([128, 128], F32, tag="d")
                nc.vector.tensor_tensor(d[:], t1[:], t2[:], mybir.AluOpType.subtract)
                nd = flush_pool.tile([128, 128], F32, tag="nd")
                nc.vector.tensor_scalar(
                    nd[:], d[:], -1.0, None, mybir.AluOpType.mult)
                a = flush_pool.tile([128, 128], F32, tag="a")
                nc.vector.tensor_tensor(a[:], d[:], nd[:], mybir.AluOpType.max)
                m = flush_pool.tile([128, 128], F32, tag="m")
                nc.vector.tensor_scalar(
                    m[:], a[:], 1.0, None, mybir.AluOpType.min)
                mh = flush_pool.tile([128, 128], F32, tag="mh")
                nc.vector.tensor_scalar(
                    mh[:], m[:], 0.5, None, mybir.AluOpType.mult)
                s = flush_pool.tile([128, 128], F32, tag="s")
                nc.vector.tensor_tensor(s[:], a[:], mh[:], mybir.AluOpType.subtract)
                h = flush_pool.tile([128, 128], F32, tag="h")
                nc.vector.tensor_tensor(h[:], m[:], s[:], mybir.AluOpType.mult)
                nc.vector.tensor_tensor(acc[:], acc[:], h[:], mybir.AluOpType.add)

        red = acc_pool.tile([128, 1], F32)
        nc.vector.tensor_reduce(red[:], acc[:], mybir.AxisListType.X, mybir.AluOpType.add)
        nc.sync.dma_start(out_part[:], red[:])

    nc.compile()
    return nc


_NC_CACHE = {}


def kernel(registration_pred, registration_gt, coords):
    coords = np.asarray(coords, dtype=np.float32)
    registration_pred = np.asarray(registration_pred, dtype=np.float32)
    registration_gt = np.asarray(registration_gt, dtype=np.float32)

    pred_pts = (coords + registration_pred).reshape(-1, 3).astype(np.float32)
    gt_pts = (coords + registration_gt).reshape(-1, 3).astype(np.float32)

    nt = NT_MIN
    for pts in (pred_pts, gt_pts):
        z0 = _pix_groups(pts)[3]
        nt = max(nt, int(-(-np.bincount(z0 + 1, minlength=129).max() // 128)))

    pred_cores = _route_points(pred_pts, nt)
    gt_cores = _route_points(gt_pts, nt)

    key = (NG, nt)
    if key not in _NC_CACHE:
        _NC_CACHE[key] = build_bass(NG, nt)
    nc = _NC_CACHE[key]

    in_maps = [
        {"recs_pred": pred_cores[c], "recs_gt": gt_cores[c]}
        for c in range(N_CORES)
    ]
    res = run_bass_kernel_spmd(nc, in_maps, list(range(N_CORES)))
    total = np.float64(0.0)
    for c in range(N_CORES):
        total += np.asarray(res.results[c]["partials"], dtype=np.float64).sum()
    return np.float32(total)


if __name__ == "__main__":
    rng = np.random.default_rng(0)
    n = 5000
    coords = rng.uniform(-0.95, 0.95, (1, n, 3)).astype(np.float32)
    rp = (0.02 * rng.standard_normal((1, n, 3))).astype(np.float32)
    rg = (0.02 * rng.standard_normal((1, n, 3))).astype(np.float32)
    print(kernel(rp, rg, coords))
